# revision 1
# baseline (speedup 1.0000x reference)
"""Trainium2 Bass kernel for nn_LocalDenseConv1D (unfold conv + BN(train) + PReLU).

Strategy: shard the 128 output positions (L) across 8 NeuronCores (16 each).
Host pre-transposes x [B,C,H,T] -> padded [H+2, C, B*T] so each core's input
slab (34 rows) is one contiguous 17.8MB region. The locally-connected
contraction is done as 96 float32r matmuls per core (K=128 = 2 tap rows x 64
channels, M=128 = 2 output positions x 64 out-channels, N=512 (b,t) columns),
accumulated in PSUM. ScalarE evicts PSUM->SBUF adding the per-(o,l) conv bias;
VectorE computes BatchNorm partial stats with bn_stats/bn_aggr; a tiny
AllGather exchanges per-core (mean, E[x^2]); each core then applies the full
BN affine + PReLU in a single ScalarE activation per tile and DMAs out.
"""
import numpy as np

import concourse.bass as bass
import concourse.tile as tile
from concourse import bacc, mybir
from concourse import bass_utils

F32 = mybir.dt.float32
F32R = mybir.dt.float32r
AF = mybir.ActivationFunctionType

N_CORES = 8
B, C, H, T = 8, 64, 256, 256
O, L = 64, 128
BT = B * T                  # 2048 moving columns total
LC = L // N_CORES           # 16 output positions per core
PAIRS = LC // 2             # 8 pairs -> M=128 matmuls
SLAB = 2 * LC + 2           # 34 tap rows per core
NT = SLAB // 2              # 17 tap-pair tiles
CW = 512                    # chunk width (max fp32 moving dim / PSUM bank)
NCH = BT // CW              # 4 chunks
BN_EPS = 1e-5
BN_N = float(B * L * T)     # population count for BN stats

_CACHE = {}


def _build_nc(reps=1, timeline=False):
    nc = bacc.Bacc(
        "TRN2",
        target_bir_lowering=False,
        debug=False,
        enable_asserts=True,
        num_devices=1 if timeline else N_CORES,
    )
    xs = nc.dram_tensor("xs", [SLAB, C, BT], F32R, kind="ExternalInput").ap()
    wb = nc.dram_tensor("wb", [3 * PAIRS, 128, 128], F32R, kind="ExternalInput").ap()
    cb = nc.dram_tensor("cb", [128, PAIRS], F32, kind="ExternalInput").ap()
    pp = nc.dram_tensor("pp", [128, 4], F32, kind="ExternalInput").ap()
    yo = nc.dram_tensor("yo", [LC, O, BT], F32, kind="ExternalOutput").ap()

    with tile.TileContext(nc) as tc:
        with (
            tc.tile_pool(name="xc", bufs=3) as xpool,
            tc.tile_pool(name="wp", bufs=1) as wpool,
            tc.tile_pool(name="yp", bufs=1) as ypool,
            tc.tile_pool(name="sp", bufs=1) as spool,
            tc.tile_pool(name="ps", bufs=8, space="PSUM") as psum,
            tc.tile_pool(name="dr", bufs=1, space="DRAM") as dram,
        ):
            for _rep in range(reps):
                wt = wpool.tile([128, 3 * PAIRS * 128], F32R)
                nc.sync.dma_start(
                    wt[:].rearrange("p (k m) -> p k m", m=128),
                    wb.rearrange("k p m -> p k m"),
                )
                cbt = spool.tile([128, PAIRS], F32)
                nc.sync.dma_start(cbt[:], cb[:])
                ppt = spool.tile([128, 4], F32)
                nc.sync.dma_start(ppt[:], pp[:])

                ysb = ypool.tile([128, PAIRS * BT], F32)
                stats = spool.tile([128, NCH * PAIRS * 6], F32)

                xsv = xs.rearrange("(t j) c n -> (j c) t n", j=2)  # [128, NT, BT]
                for cc in range(NCH):
                    xt = xpool.tile([128, NT * CW], F32R, tag="xch")
                    nc.sync.dma_start(
                        xt[:].rearrange("p (t n) -> p t n", n=CW),
                        xsv[:, :, cc * CW : (cc + 1) * CW],
                    )
                    for j in range(PAIRS):
                        pt = psum.tile([128, CW], F32, tag="acc")
                        for k in range(3):
                            mm = j * 3 + k
                            nc.tensor.matmul(
                                pt[:],
                                lhsT=wt[:, mm * 128 : (mm + 1) * 128],
                                rhs=xt[:, (2 * j + k) * CW : (2 * j + k + 1) * CW],
                                start=(k == 0),
                                stop=(k == 2),
                            )
                        ys = ysb[:, j * BT + cc * CW : j * BT + (cc + 1) * CW]
                        nc.scalar.activation(
                            ys, pt[:], AF.Identity, bias=cbt[:, j : j + 1], scale=1.0
                        )
                        si = (cc * PAIRS + j) * 6
                        nc.vector.bn_stats(stats[:, si : si + 6], ys)

                # local (mean, var) per partition -> (mean, E[x^2]) for AllGather
                mv = spool.tile([128, 2], F32)
                nc.vector.bn_aggr(mv[:], stats[:])
                agin = spool.tile([128, 2], F32)
                nc.vector.tensor_copy(agin[:, 0:1], mv[:, 0:1])
                sq = spool.tile([128, 1], F32)
                nc.vector.tensor_mul(sq[:], mv[:, 0:1], mv[:, 0:1])
                nc.vector.tensor_add(agin[:, 1:2], mv[:, 1:2], sq[:])

                agi = dram.tile([128, 2], F32)
                ago = dram.tile([N_CORES * 128, 2], F32)
                nc.sync.dma_start(agi[:], agin[:])
                if timeline:
                    for r in range(N_CORES):
                        nc.sync.dma_start(ago[r * 128 : (r + 1) * 128, :], agi[:])
                else:
                    nc.gpsimd.collective_compute(
                        "AllGather",
                        mybir.AluOpType.bypass,
                        replica_groups=[list(range(N_CORES))],
                        ins=[agi.opt()],
                        outs=[ago.opt()],
                    )
                # gather all 16 (core, half) stat pairs per channel to both halves
                g = spool.tile([128, 32], F32)
                agov = ago.rearrange("(c h o) v -> o c h v", c=N_CORES, h=2)
                for half in range(2):
                    nc.sync.dma_start(
                        g[64 * half : 64 * half + 64, :].rearrange(
                            "p (c h v) -> p c h v", c=N_CORES, h=2
                        ),
                        agov,
                    )
                red = spool.tile([128, 2], F32)
                nc.vector.tensor_reduce(
                    red[:],
                    g[:].rearrange("p (c h v) -> p v (c h)", c=N_CORES, h=2, v=2),
                    axis=mybir.AxisListType.X,
                    op=mybir.AluOpType.add,
                )
                mm2 = spool.tile([128, 2], F32)
                nc.scalar.mul(mm2[:], red[:], 1.0 / (2 * N_CORES))
                # scale = gamma * rsqrt(var+eps); shift = beta - mean*scale
                var = spool.tile([128, 1], F32)
                nc.vector.tensor_mul(var[:], mm2[:, 0:1], mm2[:, 0:1])
                nc.vector.tensor_sub(var[:], mm2[:, 1:2], var[:])
                vae = spool.tile([128, 1], F32)
                nc.vector.tensor_scalar_add(vae[:], var[:], BN_EPS)
                inv = spool.tile([128, 1], F32)
                nc.vector.reciprocal(inv[:], vae[:])
                scl = spool.tile([128, 1], F32)
                nc.scalar.sqrt(scl[:], inv[:])
                nc.vector.tensor_mul(scl[:], scl[:], ppt[:, 0:1])
                sht = spool.tile([128, 1], F32)
                nc.vector.tensor_mul(sht[:], mm2[:, 0:1], scl[:])
                nc.vector.tensor_sub(sht[:], ppt[:, 1:2], sht[:])

                yov = yo.rearrange("(pj lp) o n -> pj (lp o) n", lp=2)
                for j in range(PAIRS):
                    ys = ysb[:, j * BT : (j + 1) * BT]
                    nc.scalar.activation(
                        ys,
                        ys,
                        AF.Prelu,
                        bias=sht[:, 0:1],
                        scale=scl[:, 0:1],
                        alpha=ppt[:, 2:3],
                    )
                    nc.sync.dma_start(yov[j], ys)
    nc.compile()
    return nc


def _get_nc():
    if "nc" not in _CACHE:
        _CACHE["nc"] = _build_nc()
    return _CACHE["nc"]


def _prep_in_maps(x, weight, bias, gamma, beta, prelu_a):
    x = np.ascontiguousarray(x, dtype=np.float32)
    weight = np.asarray(weight, dtype=np.float32)
    bias = np.asarray(bias, dtype=np.float32)
    gamma = np.asarray(gamma, dtype=np.float32)
    beta = np.asarray(beta, dtype=np.float32)
    prelu_a = np.float32(np.asarray(prelu_a))

    # padded tap-row-major input: xtp[j] = x[:, :, j-1, :] as [C, B*T]
    xtp = np.zeros((H + 2, C, B, T), np.float32)
    xtp[1 : H + 1] = np.transpose(x, (2, 1, 0, 3))
    xtp = xtp.reshape(H + 2, C, BT)

    wv = weight.reshape(C, 3, O, L)  # [c, kh, o, l]
    lidx = np.arange(L).reshape(N_CORES, PAIRS, 2)
    lA, lB = lidx[:, :, 0], lidx[:, :, 1]

    def pick(kh, l2):  # -> [core, j, c, o]
        return np.transpose(wv[:, kh][:, :, l2], (2, 3, 0, 1))

    wball = np.zeros((N_CORES, PAIRS, 3, 2, C, 2, O), np.float32)
    wball[:, :, 0, 0, :, 0, :] = pick(0, lA)
    wball[:, :, 0, 1, :, 0, :] = pick(1, lA)
    wball[:, :, 1, 0, :, 0, :] = pick(2, lA)
    wball[:, :, 1, 0, :, 1, :] = pick(0, lB)
    wball[:, :, 1, 1, :, 1, :] = pick(1, lB)
    wball[:, :, 2, 0, :, 1, :] = pick(2, lB)
    wball = wball.reshape(N_CORES, 3 * PAIRS, 128, 128)

    bv = bias.reshape(O, N_CORES, PAIRS, 2)  # [o, core, j, lp]
    cball = np.ascontiguousarray(
        np.transpose(bv, (1, 3, 0, 2)).reshape(N_CORES, 128, PAIRS)
    )

    pp = np.zeros((128, 4), np.float32)
    pp[:, 0] = np.concatenate([gamma, gamma])
    pp[:, 1] = np.concatenate([beta, beta])
    pp[:, 2] = prelu_a

    in_maps = []
    for i in range(N_CORES):
        in_maps.append(
            {
                "xs": np.ascontiguousarray(xtp[32 * i : 32 * i + SLAB]),
                "wb": np.ascontiguousarray(wball[i]),
                "cb": cball[i],
                "pp": pp,
            }
        )
    return in_maps


def _unshard(results):
    outs = [
        results[i]["yo"].reshape(LC, O, B, T).transpose(2, 1, 0, 3)
        for i in range(N_CORES)
    ]
    return np.ascontiguousarray(np.concatenate(outs, axis=2), dtype=np.float32)


def kernel(x, weight, bias, gamma, beta, prelu_a):
    nc = _get_nc()
    in_maps = _prep_in_maps(x, weight, bias, gamma, beta, prelu_a)
    res = bass_utils.run_bass_kernel_spmd(
        nc, in_maps, core_ids=list(range(N_CORES)), trace=False
    )
    return _unshard(res.results)

